# revision 27
# baseline (speedup 1.0000x reference)
"""CrossCompress unit kernel for Trainium2, 8-core data parallel.

Reference computation (per batch row b, D=128):
    item_out[b]   = v[b] * (e[b]@w_vv) + e[b] * (v[b]@w_ev) + bias_v
    entity_out[b] = v[b] * (e[b]@w_ve) + e[b] * (v[b]@w_ee) + bias_e

Strategy: pure data parallel over B=16384 rows -> 2048 rows/core, worked
in a transposed layout [D=128 partitions, batch free].  Per super-tile
the four per-row dot products are PE matmuls whose stationary operand is
the (D,1) weight replicated across 128 columns (host-replicated, bf16)
-- one matmul both computes the dots AND broadcasts the result down all
partitions.  v/e stream in as bf16 (halves input DMA, ~4e-3 rel err
against a 2e-2 gate) and the matmuls run at the 1 cycle/row bf16 pump
rate.  Tile sizes [256,512,512,512,256]: a small first tile gets the PE
and DVE started ~2us earlier, a small last tile shortens the
mul->add->bias->DMA drain chain.  The combine is split by engine
capability: DVE does the wide products ([e|v] * two PSUM dot banks at
once; GPSIMD has no PSUM port), GPSIMD the cross-sums, the Scalar
engine the per-partition bias via Identity activation, writing
item/entity interleaved so each tile leaves in one output DMA.  All DMA
issues live on the otherwise idle SP queue (weights first, then inputs,
then outputs) so no compute queue ever stalls on a DMA descriptor.

Walrus CoreV3 codegen accepts only ONE embedded sync wait per
instruction; a post-pass splits any multi-wait instruction into
single-wait NoOps.
"""
import sys
sys.path.insert(0, '/opt/trn_rl_repo')
import numpy as np
import bass_rust
import concourse.bass as bass
import concourse.tile as tile
from concourse import mybir
from concourse.bass_utils import run_bass_kernel_spmd

B, D = 16384, 128
NCORES = 8
RPC = B // NCORES              # rows per core = 2048
NS = [128, 256, 512, 512, 512, 128]  # batch columns per super-tile
OFF = [0, 128, 384, 896, 1408, 1920]
assert sum(NS) == RPC

F32 = mybir.dt.float32
BF16 = mybir.dt.bfloat16
IDENT = mybir.ActivationFunctionType.Identity
ADD = mybir.AluOpType.add


def _build():
    nc = bass.Bass("TRN2", target_bir_lowering=False, debug=False,
                   num_devices=NCORES)
    # bf16 weight columns; replicated across the PE array via a stride-0
    # (broadcast) stationary access pattern -- no replicated DMA needed
    xw = nc.dram_tensor("xw", [D, 4], BF16, kind="ExternalInput").ap()
    xb = nc.dram_tensor("xb", [D, 2], F32, kind="ExternalInput").ap()
    # input stream: per tile [e | v] blocks, bf16: [D, 2*RPC]
    xin = nc.dram_tensor("xin", [D, 2 * RPC], BF16,
                         kind="ExternalInput").ap()
    # output planes (item, entity), bf16: [D, 2, RPC]
    out = nc.dram_tensor("out", [D, 2, RPC], BF16, kind="ExternalOutput").ap()

    with tile.TileContext(nc) as tc:
        with tc.tile_pool(name="c0", bufs=1) as c0_pool, \
             tc.tile_pool(name="io", bufs=len(NS)) as io_pool, \
             tc.tile_pool(name="tmp", bufs=4) as tmp_pool, \
             tc.tile_pool(name="ts", bufs=4) as ts_pool, \
             tc.tile_pool(name="o", bufs=len(NS)) as o_pool, \
             tc.tile_pool(name="ps12", bufs=2, space="PSUM") as ps12_pool, \
             tc.tile_pool(name="ps34", bufs=2, space="PSUM") as ps34_pool:

            # per-tile input DMAs up front on SP; completions pace the pipe.
            # tile0 goes first (it gates the first matmul), weights second.
            ve_tiles = []
            w_sb = c0_pool.tile([D, 4], BF16)
            c_sb = c0_pool.tile([D, 2], F32)
            last2 = len(NS) - 2
            for st, N in enumerate(NS):
                if st == last2:
                    cols = 2 * (NS[-2] + NS[-1])
                    tail_sb = io_pool.tile([D, cols], BF16, tag="vetail")
                    nc.sync.dma_start(
                        out=tail_sb[:],
                        in_=xin[:, 2 * OFF[st]:2 * OFF[st] + cols])
                    ve_tiles.append(tail_sb[:, 0:2 * NS[-2]])
                    ve_tiles.append(tail_sb[:, 2 * NS[-2]:cols])
                    break
                ve_sb = io_pool.tile([D, 2 * N], BF16, tag=f"ve{st}")
                nc.sync.dma_start(out=ve_sb[:],
                                  in_=xin[:, 2 * OFF[st]:2 * OFF[st] + 2 * N])
                ve_tiles.append(ve_sb)
                if st == 0:
                    nc.sync.dma_start(out=w_sb[:], in_=xw[:, :])
                    nc.scalar.dma_start(out=c_sb[:], in_=xb[:, :])
            w_rep = [w_sb[:, i:i + 1].broadcast_to([D, D]) for i in range(4)]
            bv_sb = c_sb[:, 0:1]
            be_sb = c_sb[:, 1:2]
            # dummy activation so the act table load runs in the idle head
            scratch = c0_pool.tile([D, 1], F32)
            nc.scalar.activation(scratch[:], bv_sb, IDENT, bias=0.0,
                                 scale=1.0)

            for st, N in enumerate(NS):
                ve_sb = ve_tiles[st]       # AP slice [D, 2N] = [e | v]
                e_b = ve_sb[:, 0:N]
                v_b = ve_sb[:, N:2 * N]

                # dot+broadcast matmuls, bf16, into two 2-bank PSUM tiles:
                #   s12 = [v@w_ev | e@w_vv] (item)   s34 = [v@w_ee | e@w_ve]
                s12 = ps12_pool.tile([D, 2, N], F32, tag="s12")
                s34 = ps34_pool.tile([D, 2, N], F32, tag="s34")
                nc.tensor.matmul(s12[:, 1], w_rep[1], e_b,
                                 start=True, stop=True)
                nc.tensor.matmul(s34[:, 1], w_rep[3], e_b,
                                 start=True, stop=True)
                nc.tensor.matmul(s12[:, 0], w_rep[0], v_b,
                                 start=True, stop=True)
                nc.tensor.matmul(s34[:, 0], w_rep[2], v_b,
                                 start=True, stop=True)

                # wide products on DVE: item pair first so GPSIMD's add can
                # start while the entity pair multiplies
                t_all = tmp_pool.tile([D, 2, 2 * N], F32, tag="tall")
                nc.vector.tensor_mul(t_all[:, 0], ve_sb[:], s12[:])
                nc.vector.tensor_mul(t_all[:, 1], ve_sb[:], s34[:])

                o_sb = o_pool.tile([D, 2, N], BF16, tag="o")
                ts_all = ts_pool.tile([D, 2, N], F32, tag="ts")
                nc.gpsimd.tensor_add(ts_all[:, 0], t_all[:, 0, 0:N],
                                     t_all[:, 0, N:2 * N])
                nc.scalar.activation(o_sb[:, 0], ts_all[:, 0], IDENT,
                                     bias=bv_sb, scale=1.0)
                nc.gpsimd.tensor_add(ts_all[:, 1], t_all[:, 1, 0:N],
                                     t_all[:, 1, N:2 * N])
                nc.scalar.activation(o_sb[:, 1], ts_all[:, 1], IDENT,
                                     bias=be_sb, scale=1.0)
                if st >= len(NS) - 2:
                    # split planes at the tail so the item half streams
                    # while the entity half is still being combined
                    nc.sync.dma_start(out=out[:, 0, OFF[st]:OFF[st] + N],
                                      in_=o_sb[:, 0])
                    nc.sync.dma_start(out=out[:, 1, OFF[st]:OFF[st] + N],
                                      in_=o_sb[:, 1])
                else:
                    nc.sync.dma_start(out=out[:, :, OFF[st]:OFF[st] + N],
                                      in_=o_sb[:])
    _split_multiwaits(nc)
    return nc


def _split_multiwaits(nc):
    """Split instructions carrying >1 sync wait into single-wait NoOps
    inserted just before them on the same engine queue."""
    n = 0
    for b in nc.m.functions[0].blocks:
        insts = b.instructions
        new = []
        for inst in insts:
            si = inst.sync_info
            if si is not None and si.on_wait and len(si.on_wait) > 1:
                waits = list(si.on_wait)
                for k, w in enumerate(waits[:-1]):
                    nop = mybir.InstNoOp(name=f"{inst.name}-sw{k}",
                                         ins=[], outs=[])
                    nop.engine = inst.engine
                    nop.sync_info = bass_rust.SyncInfo(on_wait=[w],
                                                       on_update=[])
                    nc.register_instruction(nop)
                    new.append(nop)
                    n += 1
                si.on_wait = [waits[-1]]
            new.append(inst)
        insts[:] = new
    return n


_NC = None


def _get_nc():
    global _NC
    if _NC is None:
        _NC = _build()
    return _NC


def _make_in_maps(v, e, w_vv, w_ve, w_ev, w_ee, bias_v, bias_e):
    import ml_dtypes
    bf16 = ml_dtypes.bfloat16

    xw = np.stack([w_ev.reshape(D), w_vv.reshape(D),
                   w_ee.reshape(D), w_ve.reshape(D)], axis=1).astype(bf16)
    xb = np.stack([bias_v.reshape(D), bias_e.reshape(D)],
                  axis=1).astype(np.float32)

    vT = np.ascontiguousarray(v.T).astype(bf16)   # [D, B]
    eT = np.ascontiguousarray(e.T).astype(bf16)
    in_maps = []
    for c in range(NCORES):
        xin = np.empty((D, 2 * RPC), bf16)
        base = c * RPC
        for st, N in enumerate(NS):
            lo = base + OFF[st]
            xin[:, 2 * OFF[st]:2 * OFF[st] + N] = eT[:, lo:lo + N]
            xin[:, 2 * OFF[st] + N:2 * OFF[st] + 2 * N] = vT[:, lo:lo + N]
        in_maps.append({"xw": xw, "xb": xb, "xin": xin})
    return in_maps


def _run(in_maps, trace=False):
    return run_bass_kernel_spmd(_get_nc(), in_maps, list(range(NCORES)),
                                trace=trace)


def kernel(item_embedding, entity_embedding, w_vv, w_ve, w_ev, w_ee,
           bias_v, bias_e, _trace=False, _res_out=None):
    v = np.asarray(item_embedding, np.float32).reshape(B, D)
    e = np.asarray(entity_embedding, np.float32).reshape(B, D)
    in_maps = _make_in_maps(
        v, e,
        np.asarray(w_vv, np.float32), np.asarray(w_ve, np.float32),
        np.asarray(w_ev, np.float32), np.asarray(w_ee, np.float32),
        np.asarray(bias_v, np.float32), np.asarray(bias_e, np.float32))
    res = _run(in_maps, trace=_trace)
    if _res_out is not None:
        _res_out.append(res)
    item = np.empty((B, D, 1), np.float32)
    ent = np.empty((B, D, 1), np.float32)
    for c in range(NCORES):
        o = res.results[c]["out"]            # [D, 2, RPC] bf16
        item[c * RPC:(c + 1) * RPC, :, 0] = o[:, 0].T.astype(np.float32)
        ent[c * RPC:(c + 1) * RPC, :, 0] = o[:, 1].T.astype(np.float32)
    return (item, ent)


# revision 28
# speedup vs baseline: 1.0221x; 1.0221x over previous
"""CrossCompress unit kernel for Trainium2, 8-core data parallel.

Reference computation (per batch row b, D=128):
    item_out[b]   = v[b] * (e[b]@w_vv) + e[b] * (v[b]@w_ev) + bias_v
    entity_out[b] = v[b] * (e[b]@w_ve) + e[b] * (v[b]@w_ee) + bias_e

Strategy: pure data parallel over B=16384 rows -> 2048 rows/core, worked
in a transposed layout [D=128 partitions, batch free].  Per super-tile
the four per-row dot products are PE matmuls whose stationary operand is
the (D,1) weight replicated across 128 columns (host-replicated, bf16)
-- one matmul both computes the dots AND broadcasts the result down all
partitions.  v/e stream in as bf16 (halves input DMA, ~4e-3 rel err
against a 2e-2 gate) and the matmuls run at the 1 cycle/row bf16 pump
rate.  Tile sizes [256,512,512,512,256]: a small first tile gets the PE
and DVE started ~2us earlier, a small last tile shortens the
mul->add->bias->DMA drain chain.  The combine is split by engine
capability: DVE does the wide products ([e|v] * two PSUM dot banks at
once; GPSIMD has no PSUM port), GPSIMD the cross-sums, the Scalar
engine the per-partition bias via Identity activation, writing
item/entity interleaved so each tile leaves in one output DMA.  All DMA
issues live on the otherwise idle SP queue (weights first, then inputs,
then outputs) so no compute queue ever stalls on a DMA descriptor.

Walrus CoreV3 codegen accepts only ONE embedded sync wait per
instruction; a post-pass splits any multi-wait instruction into
single-wait NoOps.
"""
import sys
sys.path.insert(0, '/opt/trn_rl_repo')
import numpy as np
import bass_rust
import concourse.bass as bass
import concourse.tile as tile
from concourse import mybir
from concourse.bass_utils import run_bass_kernel_spmd

B, D = 16384, 128
NCORES = 8
RPC = B // NCORES              # rows per core = 2048
NS = [128, 256, 512, 512, 512, 128]  # batch columns per super-tile
OFF = [0, 128, 384, 896, 1408, 1920]
assert sum(NS) == RPC

F32 = mybir.dt.float32
BF16 = mybir.dt.bfloat16
IDENT = mybir.ActivationFunctionType.Identity
ADD = mybir.AluOpType.add


def _build():
    nc = bass.Bass("TRN2", target_bir_lowering=False, debug=False,
                   num_devices=NCORES)
    # bf16 weight columns; replicated across the PE array via a stride-0
    # (broadcast) stationary access pattern -- no replicated DMA needed
    xw = nc.dram_tensor("xw", [D, 4], BF16, kind="ExternalInput").ap()
    xb = nc.dram_tensor("xb", [D, 2], F32, kind="ExternalInput").ap()
    # input stream: per tile [e | v] blocks, bf16: [D, 2*RPC]
    xin = nc.dram_tensor("xin", [D, 2 * RPC], BF16,
                         kind="ExternalInput").ap()
    # output planes (item, entity), bf16: [D, 2, RPC]
    out = nc.dram_tensor("out", [D, 2, RPC], BF16, kind="ExternalOutput").ap()

    with tile.TileContext(nc) as tc:
        with tc.tile_pool(name="c0", bufs=1) as c0_pool, \
             tc.tile_pool(name="io", bufs=len(NS)) as io_pool, \
             tc.tile_pool(name="tmp", bufs=4) as tmp_pool, \
             tc.tile_pool(name="ts", bufs=4) as ts_pool, \
             tc.tile_pool(name="o", bufs=len(NS)) as o_pool, \
             tc.tile_pool(name="ps12", bufs=2, space="PSUM") as ps12_pool, \
             tc.tile_pool(name="ps34", bufs=2, space="PSUM") as ps34_pool:

            # per-tile input DMAs up front on SP; completions pace the pipe.
            # tile0 goes first (it gates the first matmul), weights second.
            ve_tiles = []
            w_sb = c0_pool.tile([D, 4], BF16)
            c_sb = c0_pool.tile([D, 2], F32)
            for st, N in enumerate(NS):
                ve_sb = io_pool.tile([D, 2 * N], BF16, tag=f"ve{st}")
                nc.sync.dma_start(out=ve_sb[:],
                                  in_=xin[:, 2 * OFF[st]:2 * OFF[st] + 2 * N])
                ve_tiles.append(ve_sb)
                if st == 0:
                    nc.sync.dma_start(out=w_sb[:], in_=xw[:, :])
                    nc.scalar.dma_start(out=c_sb[:], in_=xb[:, :])
            w_rep = [w_sb[:, i:i + 1].broadcast_to([D, D]) for i in range(4)]
            bv_sb = c_sb[:, 0:1]
            be_sb = c_sb[:, 1:2]
            # dummy activation so the act table load runs in the idle head
            scratch = c0_pool.tile([D, 1], F32)
            nc.scalar.activation(scratch[:], bv_sb, IDENT, bias=0.0,
                                 scale=1.0)

            for st, N in enumerate(NS):
                ve_sb = ve_tiles[st]       # AP slice [D, 2N] = [e | v]
                e_b = ve_sb[:, 0:N]
                v_b = ve_sb[:, N:2 * N]

                # dot+broadcast matmuls, bf16, into two 2-bank PSUM tiles:
                #   s12 = [v@w_ev | e@w_vv] (item)   s34 = [v@w_ee | e@w_ve]
                s12 = ps12_pool.tile([D, 2, N], F32, tag="s12")
                s34 = ps34_pool.tile([D, 2, N], F32, tag="s34")
                nc.tensor.matmul(s12[:, 1], w_rep[1], e_b,
                                 start=True, stop=True)
                nc.tensor.matmul(s34[:, 1], w_rep[3], e_b,
                                 start=True, stop=True)
                nc.tensor.matmul(s12[:, 0], w_rep[0], v_b,
                                 start=True, stop=True)
                nc.tensor.matmul(s34[:, 0], w_rep[2], v_b,
                                 start=True, stop=True)

                # wide products on DVE: item pair first so GPSIMD's add can
                # start while the entity pair multiplies
                t_all = tmp_pool.tile([D, 2, 2 * N], F32, tag="tall")
                nc.vector.tensor_mul(t_all[:, 0], ve_sb[:], s12[:])
                nc.vector.tensor_mul(t_all[:, 1], ve_sb[:], s34[:])

                o_sb = o_pool.tile([D, 2, N], BF16, tag="o")
                ts_all = ts_pool.tile([D, 2, N], F32, tag="ts")
                nc.gpsimd.tensor_add(ts_all[:, 0], t_all[:, 0, 0:N],
                                     t_all[:, 0, N:2 * N])
                nc.scalar.activation(o_sb[:, 0], ts_all[:, 0], IDENT,
                                     bias=bv_sb, scale=1.0)
                nc.gpsimd.tensor_add(ts_all[:, 1], t_all[:, 1, 0:N],
                                     t_all[:, 1, N:2 * N])
                nc.scalar.activation(o_sb[:, 1], ts_all[:, 1], IDENT,
                                     bias=be_sb, scale=1.0)
                nc.sync.dma_start(out=out[:, :, OFF[st]:OFF[st] + N],
                                  in_=o_sb[:])
    _split_multiwaits(nc)
    return nc


def _split_multiwaits(nc):
    """Split instructions carrying >1 sync wait into single-wait NoOps
    inserted just before them on the same engine queue."""
    n = 0
    for b in nc.m.functions[0].blocks:
        insts = b.instructions
        new = []
        for inst in insts:
            si = inst.sync_info
            if si is not None and si.on_wait and len(si.on_wait) > 1:
                waits = list(si.on_wait)
                for k, w in enumerate(waits[:-1]):
                    nop = mybir.InstNoOp(name=f"{inst.name}-sw{k}",
                                         ins=[], outs=[])
                    nop.engine = inst.engine
                    nop.sync_info = bass_rust.SyncInfo(on_wait=[w],
                                                       on_update=[])
                    nc.register_instruction(nop)
                    new.append(nop)
                    n += 1
                si.on_wait = [waits[-1]]
            new.append(inst)
        insts[:] = new
    return n


_NC = None


def _get_nc():
    global _NC
    if _NC is None:
        _NC = _build()
    return _NC


def _make_in_maps(v, e, w_vv, w_ve, w_ev, w_ee, bias_v, bias_e):
    import ml_dtypes
    bf16 = ml_dtypes.bfloat16

    xw = np.stack([w_ev.reshape(D), w_vv.reshape(D),
                   w_ee.reshape(D), w_ve.reshape(D)], axis=1).astype(bf16)
    xb = np.stack([bias_v.reshape(D), bias_e.reshape(D)],
                  axis=1).astype(np.float32)

    vT = np.ascontiguousarray(v.T).astype(bf16)   # [D, B]
    eT = np.ascontiguousarray(e.T).astype(bf16)
    in_maps = []
    for c in range(NCORES):
        xin = np.empty((D, 2 * RPC), bf16)
        base = c * RPC
        for st, N in enumerate(NS):
            lo = base + OFF[st]
            xin[:, 2 * OFF[st]:2 * OFF[st] + N] = eT[:, lo:lo + N]
            xin[:, 2 * OFF[st] + N:2 * OFF[st] + 2 * N] = vT[:, lo:lo + N]
        in_maps.append({"xw": xw, "xb": xb, "xin": xin})
    return in_maps


def _run(in_maps, trace=False):
    return run_bass_kernel_spmd(_get_nc(), in_maps, list(range(NCORES)),
                                trace=trace)


def kernel(item_embedding, entity_embedding, w_vv, w_ve, w_ev, w_ee,
           bias_v, bias_e, _trace=False, _res_out=None):
    v = np.asarray(item_embedding, np.float32).reshape(B, D)
    e = np.asarray(entity_embedding, np.float32).reshape(B, D)
    in_maps = _make_in_maps(
        v, e,
        np.asarray(w_vv, np.float32), np.asarray(w_ve, np.float32),
        np.asarray(w_ev, np.float32), np.asarray(w_ee, np.float32),
        np.asarray(bias_v, np.float32), np.asarray(bias_e, np.float32))
    res = _run(in_maps, trace=_trace)
    if _res_out is not None:
        _res_out.append(res)
    item = np.empty((B, D, 1), np.float32)
    ent = np.empty((B, D, 1), np.float32)
    for c in range(NCORES):
        o = res.results[c]["out"]            # [D, 2, RPC] bf16
        item[c * RPC:(c + 1) * RPC, :, 0] = o[:, 0].T.astype(np.float32)
        ent[c * RPC:(c + 1) * RPC, :, 0] = o[:, 1].T.astype(np.float32)
    return (item, ent)
